# revision 2
# baseline (speedup 1.0000x reference)
"""Multi-head attention (B=2, N=2048, d_model=1024, H=16) on 8 NeuronCores.

Sharding: data-parallel on batch (2) x tensor-parallel on heads (4 groups of
4 heads). Core c handles batch c//4, head-group c%4. Each core computes its
heads' Q/K/V projections, causal attention, and a partial output projection;
the host sums the 4 partials per batch.

All matmuls run in bf16 with fp32 PSUM accumulation. Softmax skips the
max-subtraction (scores here are bounded by ~+-5, exp is safe) so attention
needs no transposes: scores are computed directly in S.T orientation
[keys, queries], exp'd, and fed to PV as the stationary operand with a
ones-column on V producing the softmax denominator for free.
"""

import sys

if "/opt/trn_rl_repo" not in sys.path:
    sys.path.insert(0, "/opt/trn_rl_repo")

import numpy as np
import ml_dtypes

import concourse.bass as bass
import concourse.mybir as mybir
import concourse.tile as tile
from concourse import bacc
from concourse.bass_utils import run_bass_kernel_spmd
from concourse.masks import make_upper_triangular

B, N, D, H = 2, 2048, 1024, 16
DV = D // H  # 64
HPC = H // 4  # heads per core: 4
DHC = HPC * DV  # head dims per core: 256
NT = N // 128  # 16 m-tiles
NC = N // 512  # 4 n-chunks
DT = D // 128  # 8 din-tiles
BF = mybir.dt.bfloat16
F32 = mybir.dt.float32
EXP = mybir.ActivationFunctionType.Exp
SCALE = 0.125  # 1/sqrt(DV)

_CACHE = {}


def build_nc():
    nc = bacc.Bacc("TRN2", target_bir_lowering=False, debug=False)
    xqT_d = nc.dram_tensor("xqT", [D, N], BF, kind="ExternalInput")
    xkT_d = nc.dram_tensor("xkT", [D, N], BF, kind="ExternalInput")
    xvT_d = nc.dram_tensor("xvT", [D, N], BF, kind="ExternalInput")
    wqT_d = nc.dram_tensor("wqT", [D, DHC], BF, kind="ExternalInput")
    woT_d = nc.dram_tensor("woT", [DHC, D], BF, kind="ExternalInput")
    bq_d = nc.dram_tensor("bq", [DHC], F32, kind="ExternalInput")
    yT_d = nc.dram_tensor("yT", [D, N], F32, kind="ExternalOutput")

    with tile.TileContext(nc) as tc:
        with (
            tc.tile_pool(name="consts", bufs=1) as consts,
            tc.tile_pool(name="xin", bufs=1) as xin,
            tc.tile_pool(name="prod", bufs=1) as prod,
            tc.tile_pool(name="work", bufs=3) as work,
            tc.tile_pool(name="norm", bufs=3) as norm,
            tc.tile_pool(name="ps", bufs=1, space="PSUM") as ps,
        ):
            # ---- constants ----
            utmask = consts.tile([128, 128], BF, name="utmask")
            make_upper_triangular(nc, utmask, val=1.0, diag=True)
            # bq as per-partition bias columns per head-pair: [128, 2]
            bq_pp = consts.tile([128, 2], F32, name="bq_pp")
            nc.sync.dma_start(
                out=bq_pp, in_=bq_d.ap().rearrange("(c p) -> p c", p=128)
            )
            # bq broadcast along free dim for the v projection: [128, DHC]
            bq_row = consts.tile([1, DHC], F32, name="bq_row")
            nc.sync.dma_start(
                out=bq_row, in_=bq_d.ap().rearrange("(a c) -> a c", a=1)
            )
            bq_bc = consts.tile([128, DHC], F32, name="bq_bc")
            nc.gpsimd.partition_broadcast(bq_bc, bq_row)

            # ---- load inputs & weights ----
            wqT = [consts.tile([128, DHC], BF, name=f"wqT{j}") for j in range(DT)]
            for j in range(DT):
                nc.sync.dma_start(
                    out=wqT[j], in_=wqT_d.ap()[j * 128 : (j + 1) * 128, :]
                )
            woT = [consts.tile([128, D], BF, name=f"woT{p}") for p in range(2)]
            for p in range(2):
                nc.sync.dma_start(
                    out=woT[p], in_=woT_d.ap()[p * 128 : (p + 1) * 128, :]
                )
            xT = {}
            for nm, dram in (("q", xqT_d), ("k", xkT_d), ("v", xvT_d)):
                for j in range(DT):
                    t = xin.tile([128, N], BF, name=f"x{nm}T{j}")
                    nc.sync.dma_start(
                        out=t, in_=dram.ap()[j * 128 : (j + 1) * 128, :]
                    )
                    xT[nm, j] = t

            # ---- phase A: projections ----
            # q.T / k.T as head-pair tensors [128, N] (2 heads x 64 rows)
            qT = [prod.tile([128, N], BF, name=f"qT{p}") for p in range(2)]
            kT = [prod.tile([128, N], BF, name=f"kT{p}") for p in range(2)]
            for nm, dst in (("q", qT), ("k", kT)):
                for p in range(2):
                    for c in range(NC):
                        pp = ps.tile(
                            [128, 512], F32, name="prj", tag="prj", bufs=2
                        )
                        for j in range(DT):
                            nc.tensor.matmul(
                                pp,
                                wqT[j][:, p * 128 : (p + 1) * 128],
                                xT[nm, j][:, c * 512 : (c + 1) * 512],
                                start=(j == 0),
                                stop=(j == DT - 1),
                            )
                        # psum -> sbuf bf16 with per-partition bias
                        nc.scalar.activation(
                            dst[p][:, c * 512 : (c + 1) * 512],
                            pp,
                            mybir.ActivationFunctionType.Identity,
                            bias=bq_pp[:, p : p + 1],
                        )
            # v with ones column: per m-tile [128, HPC, DV+1]
            vp = [
                prod.tile([128, HPC, DV + 1], BF, name=f"vp{m}")
                for m in range(NT)
            ]
            for m in range(NT):
                pv = ps.tile([128, 512], F32, name="prj_v", tag="prj", bufs=2)
                pvv = pv[:, 0:DHC]
                for j in range(DT):
                    nc.tensor.matmul(
                        pvv,
                        xT["v", j][:, m * 128 : (m + 1) * 128],
                        wqT[j],
                        start=(j == 0),
                        stop=(j == DT - 1),
                    )
                nc.vector.tensor_add(
                    vp[m][:, :, 0:DV],
                    pvv.rearrange("p (h d) -> p h d", h=HPC),
                    bq_bc.rearrange("p (h d) -> p h d", h=HPC),
                )
                nc.vector.memset(vp[m][:, :, DV : DV + 1], 1.0)

            # ---- phase B: attention ----
            # x.T (attention out) as head-pair tensors [128, N]
            xaT = [prod.tile([128, N], BF, name=f"xaT{p}") for p in range(2)]
            for h in range(HPC):
                hp, hr = divmod(h, 2)
                qh = qT[hp][hr * 64 : (hr + 1) * 64, :]
                kh = kT[hp][hr * 64 : (hr + 1) * 64, :]
                for c in range(NC):
                    jmax = 4 * c + 3
                    op = ps.tile([DV + 1, 512], F32, name="op", tag="op", bufs=2)
                    for j in range(jmax + 1):
                        off = max(0, (j - 4 * c) * 128)
                        w = 512 - off
                        sp = ps.tile(
                            [128, 512], F32, name="sp", tag="sp", bufs=3
                        )
                        nc.tensor.matmul(
                            sp[:, 0:w],
                            kh[:, j * 128 : (j + 1) * 128],
                            qh[:, c * 512 + off : (c + 1) * 512],
                            start=True,
                            stop=True,
                        )
                        pT = work.tile([128, 512], BF, name="pT", tag="pT")
                        nc.scalar.activation(
                            pT[:, 0:w], sp[:, 0:w], EXP, scale=SCALE
                        )
                        if j >= 4 * c:  # diagonal block: causal mask
                            nc.vector.tensor_mul(
                                pT[:, 0:128], pT[:, 0:128], utmask
                            )
                        nc.tensor.matmul(
                            op[:, off:512],
                            vp[j][:, h, :],
                            pT[:, 0:w],
                            start=(j == 0),
                            stop=(j == jmax),
                        )
                    # normalize: rows 0:64 /= row 64 (softmax denominator)
                    rrow = norm.tile([1, 512], F32, name="rrow", tag="rrow")
                    nc.scalar.copy(rrow, op[DV : DV + 1, :])
                    rbc = norm.tile([64, 512], F32, name="rbc", tag="rbc")
                    nc.gpsimd.partition_broadcast(rbc, rrow)
                    rrec = norm.tile([64, 512], F32, name="rrec", tag="rrec")
                    nc.vector.reciprocal(rrec, rbc)
                    nc.vector.tensor_mul(
                        xaT[hp][hr * 64 : (hr + 1) * 64, c * 512 : (c + 1) * 512],
                        op[0:DV, :],
                        rrec,
                    )

            # ---- phase C: output projection (partial; host sums groups) ----
            for t in range(DT):
                for c in range(NC):
                    yp = ps.tile([128, 512], F32, name="yp", tag="prj", bufs=2)
                    for p in range(2):
                        nc.tensor.matmul(
                            yp,
                            woT[p][:, t * 128 : (t + 1) * 128],
                            xaT[p][:, c * 512 : (c + 1) * 512],
                            start=(p == 0),
                            stop=(p == 1),
                        )
                    y_sb = work.tile([128, 512], F32, name="y_sb", tag="y_sb")
                    nc.vector.tensor_copy(y_sb, yp)
                    nc.sync.dma_start(
                        out=yT_d.ap()[
                            t * 128 : (t + 1) * 128, c * 512 : (c + 1) * 512
                        ],
                        in_=y_sb,
                    )
    nc.compile()
    return nc


def kernel(**inputs):
    Q, K, V = inputs["Q"], inputs["K"], inputs["V"]
    wq, bq, wo, bo = inputs["wq"], inputs["bq"], inputs["wo"], inputs["bo"]

    def bfT(x):  # bf16 transpose [n, d] -> [d, n]
        return np.ascontiguousarray(x.astype(ml_dtypes.bfloat16).T)

    xqT = [bfT(Q[b]) for b in range(B)]
    xkT = [bfT(K[b]) for b in range(B)]
    xvT = [bfT(V[b]) for b in range(B)]
    wqT = [bfT(wq[g * DHC : (g + 1) * DHC, :]) for g in range(4)]
    woT = [bfT(wo[:, g * DHC : (g + 1) * DHC]) for g in range(4)]
    bqs = [np.ascontiguousarray(bq[g * DHC : (g + 1) * DHC], dtype=np.float32)
           for g in range(4)]

    if "nc" not in _CACHE:
        _CACHE["nc"] = build_nc()
    nc = _CACHE["nc"]

    in_maps = []
    for core in range(8):
        b, g = divmod(core, 4)
        in_maps.append(
            {
                "xqT": xqT[b],
                "xkT": xkT[b],
                "xvT": xvT[b],
                "wqT": wqT[g],
                "woT": woT[g],
                "bq": bqs[g],
            }
        )
    import os

    trace = bool(int(os.environ.get("KERNEL_TRACE", "0")))
    res = run_bass_kernel_spmd(nc, in_maps, core_ids=list(range(8)), trace=trace)
    _CACHE["last_results"] = res

    out = np.empty((B, N, D), np.float32)
    for b in range(B):
        acc = res.results[4 * b]["yT"].astype(np.float64)
        for g in range(1, 4):
            acc += res.results[4 * b + g]["yT"]
        out[b] = acc.T + bo
    return out


# revision 4
# speedup vs baseline: 1.0773x; 1.0773x over previous
"""Multi-head attention (B=2, N=2048, d_model=1024, H=16) on 8 NeuronCores.

Sharding: data-parallel on batch (2) x tensor-parallel on heads (4 groups of
4 heads). Core c handles batch c//4, head-group c%4. Each core computes its
heads' Q/K/V projections, causal attention, and a partial output projection;
the host sums the 4 partials per batch.

All matmuls run in bf16 with fp32 PSUM accumulation. Softmax skips the
max-subtraction (scores here are bounded by ~+-5, exp is safe) so attention
needs no transposes: scores are computed directly in S.T orientation
[keys, queries], exp'd, and fed to PV as the stationary operand with a
ones-column on V producing the softmax denominator for free.

Engine budget per core: PE ~117us of matmul, ACT ~66us of exp (the softmax
exp at 1 elem/cycle/lane is the secondary bottleneck, so ACT does nothing
else on the attention path), DVE does all PSUM->SBUF copies and the
normalization, GPSIMD broadcasts the denominators.
"""

import sys

if "/opt/trn_rl_repo" not in sys.path:
    sys.path.insert(0, "/opt/trn_rl_repo")

import numpy as np
import ml_dtypes

import concourse.bass as bass
import concourse.mybir as mybir
import concourse.tile as tile
from concourse import bacc
from concourse.bass_utils import run_bass_kernel_spmd
from concourse.masks import make_upper_triangular

B, N, D, H = 2, 2048, 1024, 16
DV = D // H  # 64
HPC = H // 4  # heads per core: 4
DHC = HPC * DV  # head dims per core: 256
NT = N // 128  # 16 m-tiles
NC = N // 512  # 4 n-chunks
DT = D // 128  # 8 din-tiles
BF = mybir.dt.bfloat16
F32 = mybir.dt.float32
EXP = mybir.ActivationFunctionType.Exp
SCALE = 0.125  # 1/sqrt(DV)

_CACHE = {}


def build_nc():
    nc = bacc.Bacc("TRN2", target_bir_lowering=False, debug=False)
    xqT_d = nc.dram_tensor("xqT", [D, N], BF, kind="ExternalInput")
    xkT_d = nc.dram_tensor("xkT", [D, N], BF, kind="ExternalInput")
    xvT_d = nc.dram_tensor("xvT", [D, N], BF, kind="ExternalInput")
    wqT_d = nc.dram_tensor("wqT", [D, DHC], BF, kind="ExternalInput")
    woT_d = nc.dram_tensor("woT", [DHC, D], BF, kind="ExternalInput")
    bq_d = nc.dram_tensor("bq", [DHC], F32, kind="ExternalInput")
    yT_d = nc.dram_tensor("yT", [D, N], F32, kind="ExternalOutput")

    with tile.TileContext(nc) as tc:
        with (
            tc.tile_pool(name="consts", bufs=1) as consts,
            tc.tile_pool(name="xin", bufs=1) as xin,
            tc.tile_pool(name="prod", bufs=1) as prod,
            tc.tile_pool(name="work", bufs=3) as work,
            tc.tile_pool(name="norm", bufs=3) as norm,
            tc.tile_pool(name="yout", bufs=2) as yout,
            tc.tile_pool(name="ps", bufs=1, space="PSUM") as ps,
        ):
            # ---- weights + constants (small, load first) ----
            wqT = consts.tile([128, DT, DHC], BF, name="wqT")
            nc.sync.dma_start(
                out=wqT, in_=wqT_d.ap().rearrange("(j p) c -> p j c", p=128)
            )
            bq_pp = consts.tile([128, 2], F32, name="bq_pp")
            nc.sync.dma_start(
                out=bq_pp, in_=bq_d.ap().rearrange("(c p) -> p c", p=128)
            )
            bq_row = consts.tile([1, DHC], F32, name="bq_row")
            nc.sync.dma_start(
                out=bq_row, in_=bq_d.ap().rearrange("(a c) -> a c", a=1)
            )
            bq_bc = consts.tile([128, DHC], F32, name="bq_bc")
            nc.gpsimd.partition_broadcast(bq_bc, bq_row)
            utmask = consts.tile([128, 128], BF, name="utmask")
            make_upper_triangular(nc, utmask, val=1.0, diag=True)

            # ---- bulk inputs: one DMA each, v first (v-proj gates phase B) ----
            xvT = xin.tile([128, DT, N], BF, name="xvT")
            nc.sync.dma_start(
                out=xvT, in_=xvT_d.ap().rearrange("(j p) n -> p j n", p=128)
            )
            xkT = xin.tile([128, DT, N], BF, name="xkT")
            nc.sync.dma_start(
                out=xkT, in_=xkT_d.ap().rearrange("(j p) n -> p j n", p=128)
            )
            xqT = xin.tile([128, DT, N], BF, name="xqT")
            nc.sync.dma_start(
                out=xqT, in_=xqT_d.ap().rearrange("(j p) n -> p j n", p=128)
            )
            woT = consts.tile([128, 2, D], BF, name="woT")
            nc.sync.dma_start(
                out=woT, in_=woT_d.ap().rearrange("(q p) c -> p q c", p=128)
            )

            # ---- phase A: projections ----
            # v with ones column first: per m-tile [128, HPC, DV+1]
            vp = [
                prod.tile([128, HPC, DV + 1], BF, name=f"vp{m}")
                for m in range(NT)
            ]
            for m in range(NT):
                pv = ps.tile([128, 512], F32, name="prj_v", tag="prj", bufs=2)
                pvv = pv[:, 0:DHC]
                for j in range(DT):
                    nc.tensor.matmul(
                        pvv,
                        xvT[:, j, m * 128 : (m + 1) * 128],
                        wqT[:, j, :],
                        start=(j == 0),
                        stop=(j == DT - 1),
                    )
                nc.vector.tensor_add(
                    vp[m][:, :, 0:DV],
                    pvv.rearrange("p (h d) -> p h d", h=HPC),
                    bq_bc.rearrange("p (h d) -> p h d", h=HPC),
                )
                nc.vector.memset(vp[m][:, :, DV : DV + 1], 1.0)

            # q.T / k.T as head-pair tensors [128, N]; chunk-major so phase B
            # can start at chunk 0 for all heads ASAP
            qT = [prod.tile([128, N], BF, name=f"qT{p}") for p in range(2)]
            kT = [prod.tile([128, N], BF, name=f"kT{p}") for p in range(2)]
            for c in range(NC):
                for src, dst in ((xkT, kT), (xqT, qT)):
                    for p in range(2):
                        pp = ps.tile(
                            [128, 512], F32, name="prj_qk", tag="prj", bufs=2
                        )
                        for j in range(DT):
                            nc.tensor.matmul(
                                pp,
                                wqT[:, j, p * 128 : (p + 1) * 128],
                                src[:, j, c * 512 : (c + 1) * 512],
                                start=(j == 0),
                                stop=(j == DT - 1),
                            )
                        nc.vector.tensor_scalar_add(
                            dst[p][:, c * 512 : (c + 1) * 512],
                            pp,
                            bq_pp[:, p : p + 1],
                        )

            # ---- phase B: attention (chunk-major) + phase C per chunk ----
            xaT = [prod.tile([128, N], BF, name=f"xaT{p}") for p in range(2)]
            for c in range(NC):
                jmax = 4 * c + 3
                for h in range(HPC):
                    hp, hr = divmod(h, 2)
                    qh = qT[hp][hr * 64 : (hr + 1) * 64, :]
                    kh = kT[hp][hr * 64 : (hr + 1) * 64, :]
                    op = ps.tile([DV + 1, 512], F32, name="op", tag="op", bufs=2)
                    for j in range(jmax + 1):
                        off = max(0, (j - 4 * c) * 128)
                        w = 512 - off
                        sp = ps.tile(
                            [128, 512], F32, name="sp", tag="sp", bufs=3
                        )
                        nc.tensor.matmul(
                            sp[:, 0:w],
                            kh[:, j * 128 : (j + 1) * 128],
                            qh[:, c * 512 + off : (c + 1) * 512],
                            start=True,
                            stop=True,
                        )
                        pT = work.tile([128, 512], BF, name="pT", tag="pT")
                        nc.scalar.activation(
                            pT[:, 0:w], sp[:, 0:w], EXP, scale=SCALE
                        )
                        if j >= 4 * c:  # diagonal block: causal mask
                            nc.vector.tensor_mul(
                                pT[:, 0:128], pT[:, 0:128], utmask
                            )
                        nc.tensor.matmul(
                            op[:, off:512],
                            vp[j][:, h, :],
                            pT[:, 0:w],
                            start=(j == 0),
                            stop=(j == jmax),
                        )
                    # normalize: rows 0:64 /= row 64 (softmax denominator)
                    rrow = norm.tile([1, 512], F32, name="rrow", tag="rrow")
                    nc.scalar.copy(rrow, op[DV : DV + 1, :])
                    rbc = norm.tile([64, 512], F32, name="rbc", tag="rbc")
                    nc.gpsimd.partition_broadcast(rbc, rrow)
                    rrec = norm.tile([64, 512], F32, name="rrec", tag="rrec")
                    nc.vector.reciprocal(rrec, rbc)
                    nc.vector.tensor_mul(
                        xaT[hp][hr * 64 : (hr + 1) * 64, c * 512 : (c + 1) * 512],
                        op[0:DV, :],
                        rrec,
                    )

                # ---- phase C for this chunk: partial output projection ----
                for t in range(DT):
                    yp = ps.tile([128, 512], F32, name="yp", tag="prj", bufs=2)
                    for p in range(2):
                        nc.tensor.matmul(
                            yp,
                            woT[:, p, t * 128 : (t + 1) * 128],
                            xaT[p][:, c * 512 : (c + 1) * 512],
                            start=(p == 0),
                            stop=(p == 1),
                        )
                    y_sb = yout.tile(
                        [128, 512], F32, name=f"y_sb{t}", tag=f"y{t % 4}"
                    )
                    nc.scalar.copy(y_sb, yp)
                    nc.sync.dma_start(
                        out=yT_d.ap()[
                            t * 128 : (t + 1) * 128, c * 512 : (c + 1) * 512
                        ],
                        in_=y_sb,
                    )
    nc.compile()
    return nc


def kernel(**inputs):
    Q, K, V = inputs["Q"], inputs["K"], inputs["V"]
    wq, bq, wo, bo = inputs["wq"], inputs["bq"], inputs["wo"], inputs["bo"]

    def bfT(x):  # bf16 transpose [n, d] -> [d, n]
        return np.ascontiguousarray(x.astype(ml_dtypes.bfloat16).T)

    xqT = [bfT(Q[b]) for b in range(B)]
    xkT = [bfT(K[b]) for b in range(B)]
    xvT = [bfT(V[b]) for b in range(B)]
    wqT = [bfT(wq[g * DHC : (g + 1) * DHC, :]) for g in range(4)]
    woT = [bfT(wo[:, g * DHC : (g + 1) * DHC]) for g in range(4)]
    bqs = [np.ascontiguousarray(bq[g * DHC : (g + 1) * DHC], dtype=np.float32)
           for g in range(4)]

    if "nc" not in _CACHE:
        _CACHE["nc"] = build_nc()
    nc = _CACHE["nc"]

    in_maps = []
    for core in range(8):
        b, g = divmod(core, 4)
        in_maps.append(
            {
                "xqT": xqT[b],
                "xkT": xkT[b],
                "xvT": xvT[b],
                "wqT": wqT[g],
                "woT": woT[g],
                "bq": bqs[g],
            }
        )
    import os

    trace = bool(int(os.environ.get("KERNEL_TRACE", "0")))
    res = run_bass_kernel_spmd(nc, in_maps, core_ids=list(range(8)), trace=trace)
    _CACHE["last_results"] = res

    out = np.empty((B, N, D), np.float32)
    for b in range(B):
        acc = res.results[4 * b]["yT"].astype(np.float64)
        for g in range(1, 4):
            acc += res.results[4 * b + g]["yT"]
        out[b] = acc.T + bo
    return out
